# revision 17
# baseline (speedup 1.0000x reference)
"""Bahdanau-attention kernel for Trainium2, 8-core data-parallel.

Reference computation (per batch b):
    enc_proj = encoder_out @ W_enc            # [S, A]
    dec_proj = decoder_hidden @ W_dec         # [A]
    energy   = tanh(enc_proj + dec_proj)      # [S, A]
    scores   = energy @ v                     # [S]
    weights  = softmax(scores)                # [S]
    context  = weights @ encoder_out          # [ENC]

Sharding: batch (64) across 8 cores, 8 batches/core; W_enc/W_dec/v replicated.

Device-side layout trick: the host feeds encoder_out pre-transposed and cast to
bf16 per core as encT [8, ENC=512, S=4096] so every on-device matmul
contraction dim (ENC) lands on SBUF partitions with no on-device transposes.
All matmul inputs are bf16 (accumulation in fp32 PSUM).  The device returns
p = exp(scores) (no max-subtraction needed: |scores| <= ||v||_1 ~= 13) and the
unnormalized context accumulator ctx[e] = sum_s p[s] * enc[e, s]; the host
divides both by sum(p) to finish the softmax.
"""

import numpy as np
import ml_dtypes

import concourse.bass as bass
import concourse.tile as tile
from concourse import mybir
from concourse.bass_utils import run_bass_kernel_spmd

B, S, ENC, A = 64, 4096, 512, 256
NCORES = 8
BL = B // NCORES          # 8 batches per core
ST = 512                  # s-tile size
NST = S // ST             # 8 s-tiles
F32 = mybir.dt.float32
BF16 = mybir.dt.bfloat16
BF16_NP = ml_dtypes.bfloat16

_NC_CACHE = {}

# Engine-sem ant_name prefix -> EngineType (Tile's per-proc semaphores).
_ENGINE_SEM = {
    "PE": mybir.EngineType.PE,
    "DVE": mybir.EngineType.DVE,
    "Activation": mybir.EngineType.Activation,
    "SP": mybir.EngineType.SP,
    "Pool": mybir.EngineType.Pool,
}


def _sem_engine(ant_name):
    if ant_name is None:
        return None
    return _ENGINE_SEM.get(ant_name.rsplit("_", 1)[0])


def _legalize_waits(nc, limit=1):
    """walrus in this toolchain rejects instructions whose encoded sync-wait
    list exceeds the ISA struct's slots (and ACT instructions that wait on
    their own engine's semaphore).  Drop same-engine waits (engine FIFO
    already guarantees them in straight-line code: engine sems are only
    incremented by earlier same-engine instructions) and hoist excess waits
    onto same-engine InstNoOps inserted immediately before."""
    for bb in nc.main_func.blocks:
        new_insts = []
        for inst in bb.instructions:
            si = inst.sync_info
            if si is None or not si.on_wait:
                new_insts.append(inst)
                continue
            waits = []
            for w in si.on_wait:
                if (
                    w.sync_type == "semaphore"
                    and _sem_engine(w.ant_name) == inst.engine
                ):
                    continue
                waits.append(w)
            if len(waits) > limit:
                # keep barrier-ish waits on the original instruction
                waits.sort(key=lambda w: (w.ant_name or "").startswith("barrier"))
                excess, waits = waits[: len(waits) - limit], waits[-limit:]
                while excess:
                    chunk, excess = excess[:limit], excess[limit:]
                    new_insts.append(
                        mybir.InstNoOp(
                            name=nc.get_next_instruction_name(),
                            sync_info=mybir.SyncInfo(on_wait=chunk, on_update=[]),
                            bass_nofuse=True,
                            engine=inst.engine,
                        )
                    )
            inst.sync_info = mybir.SyncInfo(
                on_wait=waits, on_update=list(si.on_update or [])
            )
            new_insts.append(inst)
        bb.instructions[:] = new_insts


def _build_nc():
    nc = bass.Bass()

    encT = nc.declare_dram_parameter("encT", [BL, ENC, S], BF16, isOutput=False)
    decT = nc.declare_dram_parameter("decT", [ENC, BL], BF16, isOutput=False)
    wenc = nc.declare_dram_parameter("wenc", [ENC, A], BF16, isOutput=False)
    wdec = nc.declare_dram_parameter("wdec", [ENC, A], BF16, isOutput=False)
    vv = nc.declare_dram_parameter("v", [A], BF16, isOutput=False)
    p_out = nc.declare_dram_parameter("p_out", [BL, S], BF16, isOutput=True)
    # per-s-tile context partials; host sums over the NST axis
    ctx_out = nc.declare_dram_parameter("ctx_out", [BL, NST, ENC], F32, isOutput=True)

    with tile.TileContext(nc) as tc:
        with (
            tc.tile_pool(name="singles", bufs=1) as singles,
            tc.tile_pool(name="enc", bufs=3) as encpool,
            tc.tile_pool(name="energy", bufs=4) as enpool,
            tc.tile_pool(name="prow", bufs=4) as prowpool,
            tc.tile_pool(name="pbcast", bufs=3) as pbpool,
            tc.tile_pool(name="prod", bufs=2) as prodpool,
            tc.tile_pool(name="ctx", bufs=3) as ctxpool,
            tc.tile_pool(name="pe", bufs=6, space="PSUM") as psum_e,
            tc.tile_pool(name="ps", bufs=2, space="PSUM") as psum_s,
            tc.tile_pool(name="pdram", bufs=3, space="DRAM") as pdrampool,
        ):
            # ---- constants / setup --------------------------------------
            # W_enc/W_dec as [128, 4(e-chunk), 256]; ENC chunk c on rows.
            wenc_sb = singles.tile([128, 4, A], BF16)
            nc.sync.dma_start(
                out=wenc_sb, in_=wenc[:].rearrange("(c p) a -> p c a", p=128)
            )
            wdec_sb = singles.tile([128, 4, A], BF16)
            nc.sync.dma_start(
                out=wdec_sb, in_=wdec[:].rearrange("(c p) a -> p c a", p=128)
            )
            decT_sb = singles.tile([128, 4, BL], BF16)
            nc.sync.dma_start(
                out=decT_sb, in_=decT[:].rearrange("(c p) b -> p c b", p=128)
            )
            v_sb = singles.tile([128, 2], BF16)
            nc.sync.dma_start(out=v_sb, in_=vv[:].rearrange("(c p) -> p c", p=128))
            # dec_projT[a, b] = sum_d W_dec[d, a] * dec[d, b]; [128, 2(a-chunk), BL]
            dec_projT = singles.tile([128, 2, BL], F32)
            for ach in range(2):
                ps = psum_s.tile([128, BL], F32, tag="ps", name=f"dps{ach}")
                for ch in range(4):
                    nc.tensor.matmul(
                        ps,
                        lhsT=wdec_sb[:, ch, ach * 128 : (ach + 1) * 128],
                        rhs=decT_sb[:, ch, :],
                        start=(ch == 0),
                        stop=(ch == 3),
                    )
                nc.scalar.copy(dec_projT[:, ach, :], ps)

            # ---- main loop ----------------------------------------------
            for b in range(BL):
                for st in range(NST):
                    # load encT tile [128, 4(e-chunk), 512] (512 KiB,
                    # contiguous 1KB rows)
                    enc_t = encpool.tile([128, 4, ST], BF16, tag="enc")
                    nc.sync.dma_start(
                        out=enc_t,
                        in_=encT[b].rearrange("(c p) s -> p c s", p=128)[
                            :, :, st * ST : (st + 1) * ST
                        ],
                    )

                    # enc_projT psum [a-chunk 128, s 512] x2, accumulated over
                    # 4 e-chunks
                    pe_t = [
                        psum_e.tile([128, ST], F32, tag="pe", name=f"pe{i}")
                        for i in range(2)
                    ]
                    for ach in range(2):
                        for ch in range(4):
                            nc.tensor.matmul(
                                pe_t[ach],
                                lhsT=wenc_sb[:, ch, ach * 128 : (ach + 1) * 128],
                                rhs=enc_t[:, ch, :],
                                start=(ch == 0),
                                stop=(ch == 3),
                            )

                    # energy = tanh(enc_projT + dec_projT[:, b])  (bias is
                    # per-partition), bf16 out
                    en_t = [
                        enpool.tile([128, ST], BF16, tag="energy", name=f"en{i}")
                        for i in range(2)
                    ]
                    for ach in range(2):
                        nc.scalar.activation(
                            out=en_t[ach],
                            in_=pe_t[ach],
                            func=mybir.ActivationFunctionType.Tanh,
                            bias=dec_projT[:, ach, b : b + 1],
                        )

                    # scores psum [1, 512] = sum_a v[a] * energy[a, s]
                    ps_t = psum_s.tile([1, ST], F32, tag="ps")
                    for ach in range(2):
                        nc.tensor.matmul(
                            ps_t,
                            lhsT=v_sb[:, ach : ach + 1],
                            rhs=en_t[ach],
                            start=(ach == 0),
                            stop=(ach == 1),
                        )

                    # p = exp(scores) -> bf16 SBUF row, DMA straight to DRAM
                    p_row = prowpool.tile([1, ST], BF16, tag="prow")
                    nc.scalar.activation(
                        out=p_row, in_=ps_t, func=mybir.ActivationFunctionType.Exp
                    )
                    nc.sync.dma_start(
                        out=p_out[b, st * ST : (st + 1) * ST], in_=p_row
                    )

                    # broadcast p across the 128 partitions: bounce through a
                    # DRAM tile (SBUF APs cannot have a zero partition step)
                    p_dram = pdrampool.tile([1, ST], BF16, tag="pd")
                    nc.sync.dma_start(out=p_dram, in_=p_row)
                    pb_sb = pbpool.tile([128, ST], BF16, tag="pb")
                    nc.sync.dma_start(
                        out=pb_sb,
                        in_=bass.AP(
                            tensor=p_dram.tensor,
                            offset=p_dram.offset,
                            ap=[[0, 128], list(p_dram.ap[-1])],
                        ),
                    )

                    # ctx partial[e, ch] = sum_s encT[e, ch, s] * p[s]
                    prod = prodpool.tile([128, 4, ST], BF16, tag="prod")
                    nc.vector.tensor_tensor(
                        out=prod,
                        in0=enc_t,
                        in1=bass.AP(
                            tensor=pb_sb.tensor,
                            offset=pb_sb.offset,
                            ap=[list(pb_sb.ap[0]), [0, 4], list(pb_sb.ap[-1])],
                        ),
                        op=mybir.AluOpType.mult,
                    )
                    part = ctxpool.tile([128, 4], F32, tag="part")
                    nc.vector.tensor_reduce(
                        out=part,
                        in_=prod,
                        axis=mybir.AxisListType.X,
                        op=mybir.AluOpType.add,
                    )
                    # ctx partial -> DRAM (e = ch*128 + p)
                    nc.sync.dma_start(
                        out=ctx_out[b, st].rearrange("(c p) -> p c", p=128),
                        in_=part,
                    )

    _legalize_waits(nc)
    return nc


def _get_nc():
    if "nc" not in _NC_CACHE:
        _NC_CACHE["nc"] = _build_nc()
    return _NC_CACHE["nc"]


def _make_in_maps(encoder_out, decoder_hidden, W_enc, W_dec, v):
    in_maps = []
    for c in range(NCORES):
        sl = slice(c * BL, (c + 1) * BL)
        in_maps.append(
            {
                "encT": np.ascontiguousarray(
                    encoder_out[sl].transpose(0, 2, 1)
                ).astype(BF16_NP),
                "decT": np.ascontiguousarray(decoder_hidden[sl].T).astype(BF16_NP),
                "wenc": np.ascontiguousarray(W_enc).astype(BF16_NP),
                "wdec": np.ascontiguousarray(W_dec).astype(BF16_NP),
                "v": np.ascontiguousarray(v).astype(BF16_NP),
            }
        )
    return in_maps


def _postprocess(results, mask):
    p = np.concatenate([r["p_out"] for r in results], axis=0)  # [64, 4096] bf16
    # [64, NST, 512] partials -> [64, 512]
    ctx = np.concatenate([r["ctx_out"] for r in results], axis=0).sum(axis=1)
    p = np.where(np.asarray(mask), 0.0, p.astype(np.float64))
    denom = p.sum(axis=-1, keepdims=True)
    weights = (p / denom).astype(np.float32)
    context = (ctx.astype(np.float64) / denom).astype(np.float32)
    return context, weights


def kernel(encoder_out, decoder_hidden, mask, W_enc, W_dec, v):
    encoder_out = np.asarray(encoder_out, dtype=np.float32)
    decoder_hidden = np.asarray(decoder_hidden, dtype=np.float32)
    W_enc = np.asarray(W_enc, dtype=np.float32)
    W_dec = np.asarray(W_dec, dtype=np.float32)
    v = np.asarray(v, dtype=np.float32)

    nc = _get_nc()
    in_maps = _make_in_maps(encoder_out, decoder_hidden, W_enc, W_dec, v)
    res = run_bass_kernel_spmd(nc, in_maps, list(range(NCORES)))
    return _postprocess(res.results, mask)


# revision 45
# speedup vs baseline: 2.0962x; 2.0962x over previous
"""Bahdanau-attention kernel for Trainium2, 8-core data-parallel.

Reference computation (per batch b):
    enc_proj = encoder_out @ W_enc            # [S, A]
    dec_proj = decoder_hidden @ W_dec         # [A]
    energy   = tanh(enc_proj + dec_proj)      # [S, A]
    scores   = energy @ v                     # [S]
    weights  = softmax(scores)                # [S]
    context  = weights @ encoder_out          # [ENC]

Sharding: batch (64) across 8 cores, 8 batches/core; W_enc/W_dec/v replicated.

Device-side layout trick: the host feeds encoder_out pre-transposed and cast to
bf16 per core as encT [8, ENC=512, S=4096] so every on-device matmul
contraction dim (ENC) lands on SBUF partitions with no on-device transposes.
All matmul inputs are bf16 (accumulation in fp32 PSUM).  The device returns
p = exp(scores) (no max-subtraction needed: |scores| <= ||v||_1 ~= 13) and the
unnormalized context accumulator ctx[e] = sum_s p[s] * enc[e, s]; the host
divides both by sum(p) to finish the softmax.
"""

import numpy as np
import ml_dtypes

import concourse.bass as bass
import concourse.tile as tile
from concourse import mybir
from concourse.bass_utils import run_bass_kernel_spmd

B, S, ENC, A = 64, 4096, 512, 256
NCORES = 8
BL = B // NCORES          # 8 batches per core
ST = 512                  # s-tile size
NST = S // ST             # 8 s-tiles
F32 = mybir.dt.float32
BF16 = mybir.dt.bfloat16
BF16_NP = ml_dtypes.bfloat16

_NC_CACHE = {}

# Engine-sem ant_name prefix -> EngineType (Tile's per-proc semaphores).
_ENGINE_SEM = {
    "PE": mybir.EngineType.PE,
    "DVE": mybir.EngineType.DVE,
    "Activation": mybir.EngineType.Activation,
    "SP": mybir.EngineType.SP,
    "Pool": mybir.EngineType.Pool,
}


def _sem_engine(ant_name):
    if ant_name is None:
        return None
    return _ENGINE_SEM.get(ant_name.rsplit("_", 1)[0])


def _legalize_waits(nc, limit=1):
    """walrus in this toolchain rejects instructions whose encoded sync-wait
    list exceeds the ISA struct's slots (and ACT instructions that wait on
    their own engine's semaphore).  Drop same-engine waits (engine FIFO
    already guarantees them in straight-line code: engine sems are only
    incremented by earlier same-engine instructions) and hoist excess waits
    onto same-engine InstNoOps inserted immediately before."""
    for bb in nc.main_func.blocks:
        new_insts = []
        for inst in bb.instructions:
            si = inst.sync_info
            if si is None or not si.on_wait:
                new_insts.append(inst)
                continue
            waits = []
            for w in si.on_wait:
                if (
                    w.sync_type == "semaphore"
                    and _sem_engine(w.ant_name) == inst.engine
                ):
                    continue
                waits.append(w)
            if len(waits) > limit:
                # keep barrier-ish waits on the original instruction
                waits.sort(key=lambda w: (w.ant_name or "").startswith("barrier"))
                excess, waits = waits[: len(waits) - limit], waits[-limit:]
                while excess:
                    chunk, excess = excess[:limit], excess[limit:]
                    new_insts.append(
                        mybir.InstNoOp(
                            name=nc.get_next_instruction_name(),
                            sync_info=mybir.SyncInfo(on_wait=chunk, on_update=[]),
                            bass_nofuse=True,
                            engine=inst.engine,
                        )
                    )
            inst.sync_info = mybir.SyncInfo(
                on_wait=waits, on_update=list(si.on_update or [])
            )
            new_insts.append(inst)
        bb.instructions[:] = new_insts


def _flush_ctx_tail(nc, prodpool, ctxpool, trashpool, ctx_out, pending, HL):
    """Context multiply-reduce for a finished half-batch: DVE handles chunks
    0-2, gpsimd multiplies chunk 3 with the ACT accumulate-reduce, then the
    partial is written back."""
    enc_B, pb_h, b, hf = pending
    enc_h = enc_B[:, :, hf * HL : (hf + 1) * HL]
    prod = prodpool.tile([128, 4, HL], BF16, tag="prod", name="prod")
    nc.vector.tensor_tensor(
        out=prod,
        in0=enc_h,
        in1=bass.AP(
            tensor=pb_h.tensor,
            offset=pb_h.offset,
            ap=[list(pb_h.ap[0]), [0, 4], list(pb_h.ap[-1])],
        ),
        op=mybir.AluOpType.mult,
    )
    part = ctxpool.tile([128, 4], F32, tag="part", name="part")
    nc.vector.tensor_reduce(
        out=part[:, 0:3],
        in_=prod[:, 0:3, :],
        axis=mybir.AxisListType.X,
        op=mybir.AluOpType.add,
    )
    trash = trashpool.tile([128, HL], BF16, tag="trash", name="trash")
    nc.scalar.activation(
        out=trash,
        in_=prod[:, 3, :],
        func=mybir.ActivationFunctionType.Copy,
        accum_out=part[:, 3:4],
    )
    nc.gpsimd.dma_start(
        out=ctx_out[b, hf].rearrange("(c p) -> p c", p=128),
        in_=part,
    )


def _build_nc():
    nc = bass.Bass()

    encT = nc.declare_dram_parameter("encT", [BL, ENC, S], BF16, isOutput=False)
    decT = nc.declare_dram_parameter("decT", [ENC, BL], BF16, isOutput=False)
    wenc = nc.declare_dram_parameter("wenc", [ENC, A], BF16, isOutput=False)
    wdec = nc.declare_dram_parameter("wdec", [ENC, A], BF16, isOutput=False)
    vv = nc.declare_dram_parameter("v", [A], BF16, isOutput=False)
    p_out = nc.declare_dram_parameter("p_out", [BL, S], BF16, isOutput=True)
    # per-half-batch context partials; host sums over axis 1
    ctx_out = nc.declare_dram_parameter("ctx_out", [BL, 2, ENC], F32, isOutput=True)

    with tile.TileContext(nc) as tc:
        with (
            tc.tile_pool(name="singles", bufs=1) as singles,
            tc.tile_pool(name="enc", bufs=2) as encpool,
            tc.tile_pool(name="energy", bufs=6) as enpool,
            tc.tile_pool(name="prow", bufs=4) as prowpool,
            tc.tile_pool(name="pbcast", bufs=3) as pbpool,
            tc.tile_pool(name="prod", bufs=3) as prodpool,
            tc.tile_pool(name="trash", bufs=2) as trashpool,
            tc.tile_pool(name="ctx", bufs=6) as ctxpool,
            tc.tile_pool(name="pe", bufs=6, space="PSUM") as psum_e,
            tc.tile_pool(name="ps", bufs=2, space="PSUM") as psum_s,
            tc.tile_pool(name="pdram", bufs=2, space="DRAM") as pdrampool,
        ):
            # ---- constants / setup --------------------------------------
            # W_enc/W_dec as [128, 4(e-chunk), 256]; ENC chunk c on rows.
            wenc_sb = singles.tile([128, 4, A], BF16)
            nc.sync.dma_start(
                out=wenc_sb, in_=wenc[:].rearrange("(c p) a -> p c a", p=128)
            )
            wdec_sb = singles.tile([128, 4, A], BF16)
            nc.sync.dma_start(
                out=wdec_sb, in_=wdec[:].rearrange("(c p) a -> p c a", p=128)
            )
            decT_sb = singles.tile([128, 4, BL], BF16)
            nc.sync.dma_start(
                out=decT_sb, in_=decT[:].rearrange("(c p) b -> p c b", p=128)
            )
            v_sb = singles.tile([128, 2], BF16)
            nc.sync.dma_start(out=v_sb, in_=vv[:].rearrange("(c p) -> p c", p=128))
            # dec_projT[a, b] = sum_d W_dec[d, a] * dec[d, b]; [128, 2(a-chunk), BL]
            dec_projT = singles.tile([128, 2, BL], F32)
            for ach in range(2):
                ps = psum_s.tile([128, BL], F32, tag="ps", name=f"dps{ach}")
                for ch in range(4):
                    nc.tensor.matmul(
                        ps,
                        lhsT=wdec_sb[:, ch, ach * 128 : (ach + 1) * 128],
                        rhs=decT_sb[:, ch, :],
                        start=(ch == 0),
                        stop=(ch == 3),
                    )
                nc.scalar.copy(dec_projT[:, ach, :], ps)

            # ---- main loop ----------------------------------------------
            # Work is organized per batch in half-batches ("hf", 2048
            # s-positions): scores/exp stream per 512-s unit; p broadcast and
            # the context multiply-reduce run at half-batch granularity so the
            # DVE/ACT context work starts mid-batch and per-op overheads
            # amortize over larger tiles.
            HL = S // 2  # half-batch s length (2048)
            pending = None
            for b in range(BL):
                # one encT tile per batch [128, 4(e-chunk), 4096], loaded by 4
                # sliced DMAs (1 MiB each) so compute starts after the first
                # slice; held until the ctx phase
                enc_B = encpool.tile([128, 4, S], BF16, tag="enc")
                encT_r = encT[b].rearrange("(c p) s -> p c s", p=128)
                for q in range(4):
                    qs = slice(q * (S // 4), (q + 1) * (S // 4))
                    nc.sync.dma_start(out=enc_B[:, :, qs], in_=encT_r[:, :, qs])

                p_half = [None, None]
                for hf in range(2):
                    enc_t = enc_B[:, :, hf * HL : (hf + 1) * HL]
                    p_half[hf] = prowpool.tile(
                        [1, HL], BF16, tag="prow", name=f"ph{hf}"
                    )

                    for u in range(HL // ST):  # 4 units of 512 s
                        st = hf * (HL // ST) + u
                        sl = slice(u * ST, (u + 1) * ST)
                        # enc_projT psum [a-chunk 128, s 512] x2, accumulated
                        # over 4 e-chunks
                        pe_t = [
                            psum_e.tile([128, ST], F32, tag="pe", name=f"pe{i}")
                            for i in range(2)
                        ]
                        for ach in range(2):
                            for ch in range(4):
                                nc.tensor.matmul(
                                    pe_t[ach],
                                    lhsT=wenc_sb[
                                        :, ch, ach * 128 : (ach + 1) * 128
                                    ],
                                    rhs=enc_t[:, ch, sl],
                                    start=(ch == 0),
                                    stop=(ch == 3),
                                )

                        # energy = tanh(enc_projT + dec_projT[:, b]) (bias is
                        # per-partition), bf16 out
                        en_t = [
                            enpool.tile([128, ST], BF16, tag="energy", name=f"en{i}")
                            for i in range(2)
                        ]
                        for ach in range(2):
                            nc.scalar.activation(
                                out=en_t[ach],
                                in_=pe_t[ach],
                                func=mybir.ActivationFunctionType.Tanh,
                                bias=dec_projT[:, ach, b : b + 1],
                            )

                        # scores psum [1, 512] = sum_a v[a] * energy[a, s]
                        ps_t = psum_s.tile([1, ST], F32, tag="ps")
                        for ach in range(2):
                            nc.tensor.matmul(
                                ps_t,
                                lhsT=v_sb[:, ach : ach + 1],
                                rhs=en_t[ach],
                                start=(ach == 0),
                                stop=(ach == 1),
                            )

                        # p = exp(scores), collected into the half-batch row
                        nc.scalar.activation(
                            out=p_half[hf][0:1, sl],
                            in_=ps_t,
                            func=mybir.ActivationFunctionType.Exp,
                        )

                    # p handling for this half: output write + broadcast
                    # across the 128 partitions (bounced through a DRAM tile;
                    # SBUF APs cannot have a zero partition step)
                    nc.gpsimd.dma_start(
                        out=p_out[b, hf * HL : (hf + 1) * HL], in_=p_half[hf]
                    )
                    p_dram = pdrampool.tile([1, HL], BF16, tag="pd")
                    nc.gpsimd.dma_start(out=p_dram, in_=p_half[hf])
                    pb_h = pbpool.tile([128, HL], BF16, tag="pb")
                    nc.sync.dma_start(
                        out=pb_h,
                        in_=bass.AP(
                            tensor=p_dram.tensor,
                            offset=p_dram.offset,
                            ap=[[0, 128], list(p_dram.ap[-1])],
                        ),
                    )

                    # ctx partial for this half: [e, ch] = sum_s enc*p.
                    # The whole multiply-reduce is software-pipelined one
                    # half-batch late so no engine's in-order queue stalls the
                    # next half's work waiting on the broadcast chain.
                    if pending is not None:
                        _flush_ctx_tail(
                            nc, prodpool, ctxpool, trashpool, ctx_out, pending, HL
                        )
                    pending = (enc_B, pb_h, b, hf)

            # flush the last half's pipelined tail
            _flush_ctx_tail(nc, prodpool, ctxpool, trashpool, ctx_out, pending, HL)

    _legalize_waits(nc)
    return nc


def _get_nc():
    if "nc" not in _NC_CACHE:
        _NC_CACHE["nc"] = _build_nc()
    return _NC_CACHE["nc"]


def _make_in_maps(encoder_out, decoder_hidden, W_enc, W_dec, v):
    in_maps = []
    for c in range(NCORES):
        sl = slice(c * BL, (c + 1) * BL)
        in_maps.append(
            {
                "encT": np.ascontiguousarray(
                    encoder_out[sl].transpose(0, 2, 1)
                ).astype(BF16_NP),
                "decT": np.ascontiguousarray(decoder_hidden[sl].T).astype(BF16_NP),
                "wenc": np.ascontiguousarray(W_enc).astype(BF16_NP),
                "wdec": np.ascontiguousarray(W_dec).astype(BF16_NP),
                "v": np.ascontiguousarray(v).astype(BF16_NP),
            }
        )
    return in_maps


def _postprocess(results, mask):
    p = np.concatenate([r["p_out"] for r in results], axis=0)  # [64, 4096] bf16
    ctx = np.concatenate([r["ctx_out"] for r in results], axis=0).sum(axis=1)
    p = np.where(np.asarray(mask), 0.0, p.astype(np.float64))
    denom = p.sum(axis=-1, keepdims=True)
    weights = (p / denom).astype(np.float32)
    context = (ctx.astype(np.float64) / denom).astype(np.float32)
    return context, weights


def kernel(encoder_out, decoder_hidden, mask, W_enc, W_dec, v):
    encoder_out = np.asarray(encoder_out, dtype=np.float32)
    decoder_hidden = np.asarray(decoder_hidden, dtype=np.float32)
    W_enc = np.asarray(W_enc, dtype=np.float32)
    W_dec = np.asarray(W_dec, dtype=np.float32)
    v = np.asarray(v, dtype=np.float32)

    nc = _get_nc()
    in_maps = _make_in_maps(encoder_out, decoder_hidden, W_enc, W_dec, v)
    res = run_bass_kernel_spmd(nc, in_maps, list(range(NCORES)))
    return _postprocess(res.results, mask)


# revision 53
# speedup vs baseline: 2.2342x; 1.0658x over previous
"""Bahdanau-attention kernel for Trainium2, 8-core data-parallel.

Reference computation (per batch b):
    enc_proj = encoder_out @ W_enc            # [S, A]
    dec_proj = decoder_hidden @ W_dec         # [A]
    energy   = tanh(enc_proj + dec_proj)      # [S, A]
    scores   = energy @ v                     # [S]
    weights  = softmax(scores)                # [S]
    context  = weights @ encoder_out          # [ENC]

Sharding: batch (64) across 8 cores, 8 batches/core; W_enc/W_dec/v replicated.

Device-side layout trick: the host feeds encoder_out pre-transposed and cast to
bf16 per core as encT [8, ENC=512, S=4096] so every on-device matmul
contraction dim (ENC) lands on SBUF partitions with no on-device transposes.
All matmul inputs are bf16 (accumulation in fp32 PSUM).  The device returns
p = exp(scores) (no max-subtraction needed: |scores| <= ||v||_1 ~= 13) and the
unnormalized context accumulator ctx[e] = sum_s p[s] * enc[e, s]; the host
divides both by sum(p) to finish the softmax.
"""

import numpy as np
import ml_dtypes

import concourse.bass as bass
import concourse.tile as tile
from concourse import mybir
from concourse.bass_utils import run_bass_kernel_spmd

B, S, ENC, A = 64, 4096, 512, 256
NCORES = 8
BL = B // NCORES          # 8 batches per core
ST = 512                  # s-tile size
NST = S // ST             # 8 s-tiles
F32 = mybir.dt.float32
BF16 = mybir.dt.bfloat16
BF16_NP = ml_dtypes.bfloat16

_NC_CACHE = {}

# Engine-sem ant_name prefix -> EngineType (Tile's per-proc semaphores).
_ENGINE_SEM = {
    "PE": mybir.EngineType.PE,
    "DVE": mybir.EngineType.DVE,
    "Activation": mybir.EngineType.Activation,
    "SP": mybir.EngineType.SP,
    "Pool": mybir.EngineType.Pool,
}


def _sem_engine(ant_name):
    if ant_name is None:
        return None
    return _ENGINE_SEM.get(ant_name.rsplit("_", 1)[0])


def _legalize_waits(nc, limit=1):
    """walrus in this toolchain rejects instructions whose encoded sync-wait
    list exceeds the ISA struct's slots (and ACT instructions that wait on
    their own engine's semaphore).  Drop same-engine waits (engine FIFO
    already guarantees them in straight-line code: engine sems are only
    incremented by earlier same-engine instructions) and hoist excess waits
    onto same-engine InstNoOps inserted immediately before."""
    for bb in nc.main_func.blocks:
        new_insts = []
        for inst in bb.instructions:
            si = inst.sync_info
            if si is None or not si.on_wait:
                new_insts.append(inst)
                continue
            waits = []
            for w in si.on_wait:
                if (
                    w.sync_type == "semaphore"
                    and _sem_engine(w.ant_name) == inst.engine
                ):
                    continue
                waits.append(w)
            if len(waits) > limit:
                # keep barrier-ish waits on the original instruction
                waits.sort(key=lambda w: (w.ant_name or "").startswith("barrier"))
                excess, waits = waits[: len(waits) - limit], waits[-limit:]
                while excess:
                    chunk, excess = excess[:limit], excess[limit:]
                    new_insts.append(
                        mybir.InstNoOp(
                            name=nc.get_next_instruction_name(),
                            sync_info=mybir.SyncInfo(on_wait=chunk, on_update=[]),
                            bass_nofuse=True,
                            engine=inst.engine,
                        )
                    )
            inst.sync_info = mybir.SyncInfo(
                on_wait=waits, on_update=list(si.on_update or [])
            )
            new_insts.append(inst)
        bb.instructions[:] = new_insts


def _flush_ctx_tail(nc, prodpool, ctxpool, trashpool, ctx_out, pending, HL):
    """Context multiply-reduce for a finished half-batch.

    prod = enc * p on DVE (2x bf16 mode), then two pairwise tree-fold ADDs
    (also 2x mode) shrink the summation input 4x before the 1-elem/cycle
    tensor_reduce; chunk 3's tail is summed on ACT (Copy + accum_out) from the
    once-folded tensor to balance the engines."""
    enc_B, pb_h, b, hf = pending
    enc_h = enc_B[:, :, hf * HL : (hf + 1) * HL]
    prod = prodpool.tile([128, 4, HL], BF16, tag="prod", name="prod")
    nc.vector.tensor_tensor(
        out=prod,
        in0=enc_h,
        in1=bass.AP(
            tensor=pb_h.tensor,
            offset=pb_h.offset,
            ap=[list(pb_h.ap[0]), [0, 4], list(pb_h.ap[-1])],
        ),
        op=mybir.AluOpType.mult,
    )
    f1 = prodpool.tile([128, 4, HL // 2], BF16, tag="fold1", name="f1")
    nc.vector.tensor_tensor(
        out=f1[:, 0:3, :],
        in0=prod[:, 0:3, 0 : HL // 2],
        in1=prod[:, 0:3, HL // 2 : HL],
        op=mybir.AluOpType.add,
    )
    nc.gpsimd.tensor_tensor(
        out=f1[:, 3, :],
        in0=prod[:, 3, 0 : HL // 2],
        in1=prod[:, 3, HL // 2 : HL],
        op=mybir.AluOpType.add,
    )
    f2 = prodpool.tile([128, 3, HL // 4], BF16, tag="fold2", name="f2")
    nc.vector.tensor_tensor(
        out=f2,
        in0=f1[:, 0:3, 0 : HL // 4],
        in1=f1[:, 0:3, HL // 4 : HL // 2],
        op=mybir.AluOpType.add,
    )
    f3 = prodpool.tile([128, 3, HL // 8], BF16, tag="fold3", name="f3")
    nc.vector.tensor_tensor(
        out=f3,
        in0=f2[:, :, 0 : HL // 8],
        in1=f2[:, :, HL // 8 : HL // 4],
        op=mybir.AluOpType.add,
    )
    part = ctxpool.tile([128, 4], F32, tag="part", name="part")
    nc.vector.tensor_reduce(
        out=part[:, 0:3],
        in_=f3,
        axis=mybir.AxisListType.X,
        op=mybir.AluOpType.add,
    )
    trash = trashpool.tile([128, HL // 2], BF16, tag="trash", name="trash")
    nc.scalar.activation(
        out=trash,
        in_=f1[:, 3, :],
        func=mybir.ActivationFunctionType.Copy,
        accum_out=part[:, 3:4],
    )
    nc.gpsimd.dma_start(
        out=ctx_out[b, hf].rearrange("(c p) -> p c", p=128),
        in_=part,
    )


def _build_nc():
    nc = bass.Bass()

    encT = nc.declare_dram_parameter("encT", [BL, ENC, S], BF16, isOutput=False)
    decT = nc.declare_dram_parameter("decT", [ENC, BL], BF16, isOutput=False)
    wenc = nc.declare_dram_parameter("wenc", [ENC, A], BF16, isOutput=False)
    wdec = nc.declare_dram_parameter("wdec", [ENC, A], BF16, isOutput=False)
    vv = nc.declare_dram_parameter("v", [A], BF16, isOutput=False)
    p_out = nc.declare_dram_parameter("p_out", [BL, S], BF16, isOutput=True)
    # per-half-batch context partials; host sums over axis 1
    ctx_out = nc.declare_dram_parameter("ctx_out", [BL, 2, ENC], F32, isOutput=True)

    with tile.TileContext(nc) as tc:
        with (
            tc.tile_pool(name="singles", bufs=1) as singles,
            tc.tile_pool(name="enc", bufs=2) as encpool,
            tc.tile_pool(name="energy", bufs=6) as enpool,
            tc.tile_pool(name="prow", bufs=4) as prowpool,
            tc.tile_pool(name="pbcast", bufs=3) as pbpool,
            tc.tile_pool(name="prod", bufs=2) as prodpool,
            tc.tile_pool(name="trash", bufs=2) as trashpool,
            tc.tile_pool(name="ctx", bufs=6) as ctxpool,
            tc.tile_pool(name="pe", bufs=6, space="PSUM") as psum_e,
            tc.tile_pool(name="ps", bufs=2, space="PSUM") as psum_s,
            tc.tile_pool(name="pdram", bufs=2, space="DRAM") as pdrampool,
        ):
            # ---- constants / setup --------------------------------------
            # W_enc/W_dec as [128, 4(e-chunk), 256]; ENC chunk c on rows.
            wenc_sb = singles.tile([128, 4, A], BF16)
            nc.sync.dma_start(
                out=wenc_sb, in_=wenc[:].rearrange("(c p) a -> p c a", p=128)
            )
            wdec_sb = singles.tile([128, 4, A], BF16)
            nc.sync.dma_start(
                out=wdec_sb, in_=wdec[:].rearrange("(c p) a -> p c a", p=128)
            )
            decT_sb = singles.tile([128, 4, BL], BF16)
            nc.sync.dma_start(
                out=decT_sb, in_=decT[:].rearrange("(c p) b -> p c b", p=128)
            )
            v_sb = singles.tile([128, 2], BF16)
            nc.sync.dma_start(out=v_sb, in_=vv[:].rearrange("(c p) -> p c", p=128))
            # dec_projT[a, b] = sum_d W_dec[d, a] * dec[d, b]; [128, 2(a-chunk), BL]
            dec_projT = singles.tile([128, 2, BL], F32)
            for ach in range(2):
                ps = psum_s.tile([128, BL], F32, tag="ps", name=f"dps{ach}")
                for ch in range(4):
                    nc.tensor.matmul(
                        ps,
                        lhsT=wdec_sb[:, ch, ach * 128 : (ach + 1) * 128],
                        rhs=decT_sb[:, ch, :],
                        start=(ch == 0),
                        stop=(ch == 3),
                    )
                nc.scalar.copy(dec_projT[:, ach, :], ps)

            # ---- main loop ----------------------------------------------
            # Work is organized per batch in half-batches ("hf", 2048
            # s-positions): scores/exp stream per 512-s unit; p broadcast and
            # the context multiply-reduce run at half-batch granularity so the
            # DVE/ACT context work starts mid-batch and per-op overheads
            # amortize over larger tiles.
            HL = S // 2  # half-batch s length (2048)
            pending = None
            for b in range(BL):
                # one encT tile per batch [128, 4(e-chunk), 4096], loaded by 4
                # sliced DMAs (1 MiB each) so compute starts after the first
                # slice; held until the ctx phase
                enc_B = encpool.tile([128, 4, S], BF16, tag="enc")
                encT_r = encT[b].rearrange("(c p) s -> p c s", p=128)
                for q in range(4):
                    qs = slice(q * (S // 4), (q + 1) * (S // 4))
                    nc.sync.dma_start(out=enc_B[:, :, qs], in_=encT_r[:, :, qs])

                p_half = [None, None]
                for hf in range(2):
                    enc_t = enc_B[:, :, hf * HL : (hf + 1) * HL]
                    p_half[hf] = prowpool.tile(
                        [1, HL], BF16, tag="prow", name=f"ph{hf}"
                    )

                    for u in range(HL // ST):  # 4 units of 512 s
                        st = hf * (HL // ST) + u
                        sl = slice(u * ST, (u + 1) * ST)
                        # enc_projT psum [a-chunk 128, s 512] x2, accumulated
                        # over 4 e-chunks
                        pe_t = [
                            psum_e.tile([128, ST], F32, tag="pe", name=f"pe{i}")
                            for i in range(2)
                        ]
                        for ach in range(2):
                            for ch in range(4):
                                nc.tensor.matmul(
                                    pe_t[ach],
                                    lhsT=wenc_sb[
                                        :, ch, ach * 128 : (ach + 1) * 128
                                    ],
                                    rhs=enc_t[:, ch, sl],
                                    start=(ch == 0),
                                    stop=(ch == 3),
                                )

                        # energy = tanh(enc_projT + dec_projT[:, b]) (bias is
                        # per-partition), bf16 out
                        en_t = [
                            enpool.tile([128, ST], BF16, tag="energy", name=f"en{i}")
                            for i in range(2)
                        ]
                        for ach in range(2):
                            nc.scalar.activation(
                                out=en_t[ach],
                                in_=pe_t[ach],
                                func=mybir.ActivationFunctionType.Tanh,
                                bias=dec_projT[:, ach, b : b + 1],
                            )

                        # scores psum [1, 512] = sum_a v[a] * energy[a, s]
                        ps_t = psum_s.tile([1, ST], F32, tag="ps")
                        for ach in range(2):
                            nc.tensor.matmul(
                                ps_t,
                                lhsT=v_sb[:, ach : ach + 1],
                                rhs=en_t[ach],
                                start=(ach == 0),
                                stop=(ach == 1),
                            )

                        # p = exp(scores), collected into the half-batch row
                        nc.scalar.activation(
                            out=p_half[hf][0:1, sl],
                            in_=ps_t,
                            func=mybir.ActivationFunctionType.Exp,
                        )

                    # p handling for this half: output write + broadcast
                    # across the 128 partitions (bounced through a DRAM tile;
                    # SBUF APs cannot have a zero partition step)
                    nc.gpsimd.dma_start(
                        out=p_out[b, hf * HL : (hf + 1) * HL], in_=p_half[hf]
                    )
                    p_dram = pdrampool.tile([1, HL], BF16, tag="pd")
                    nc.gpsimd.dma_start(out=p_dram, in_=p_half[hf])
                    pb_h = pbpool.tile([128, HL], BF16, tag="pb")
                    nc.sync.dma_start(
                        out=pb_h,
                        in_=bass.AP(
                            tensor=p_dram.tensor,
                            offset=p_dram.offset,
                            ap=[[0, 128], list(p_dram.ap[-1])],
                        ),
                    )

                    # ctx partial for this half: [e, ch] = sum_s enc*p.
                    # The whole multiply-reduce is software-pipelined one
                    # half-batch late so no engine's in-order queue stalls the
                    # next half's work waiting on the broadcast chain.
                    if pending is not None:
                        _flush_ctx_tail(
                            nc, prodpool, ctxpool, trashpool, ctx_out, pending, HL
                        )
                    pending = (enc_B, pb_h, b, hf)

            # flush the last half's pipelined tail
            _flush_ctx_tail(nc, prodpool, ctxpool, trashpool, ctx_out, pending, HL)

    _legalize_waits(nc)
    return nc


def _get_nc():
    if "nc" not in _NC_CACHE:
        _NC_CACHE["nc"] = _build_nc()
    return _NC_CACHE["nc"]


def _make_in_maps(encoder_out, decoder_hidden, W_enc, W_dec, v):
    in_maps = []
    for c in range(NCORES):
        sl = slice(c * BL, (c + 1) * BL)
        in_maps.append(
            {
                "encT": np.ascontiguousarray(
                    encoder_out[sl].transpose(0, 2, 1)
                ).astype(BF16_NP),
                "decT": np.ascontiguousarray(decoder_hidden[sl].T).astype(BF16_NP),
                "wenc": np.ascontiguousarray(W_enc).astype(BF16_NP),
                "wdec": np.ascontiguousarray(W_dec).astype(BF16_NP),
                "v": np.ascontiguousarray(v).astype(BF16_NP),
            }
        )
    return in_maps


def _postprocess(results, mask):
    p = np.concatenate([r["p_out"] for r in results], axis=0)  # [64, 4096] bf16
    ctx = np.concatenate([r["ctx_out"] for r in results], axis=0).sum(axis=1)
    p = np.where(np.asarray(mask), 0.0, p.astype(np.float64))
    denom = p.sum(axis=-1, keepdims=True)
    weights = (p / denom).astype(np.float32)
    context = (ctx.astype(np.float64) / denom).astype(np.float32)
    return context, weights


def kernel(encoder_out, decoder_hidden, mask, W_enc, W_dec, v):
    encoder_out = np.asarray(encoder_out, dtype=np.float32)
    decoder_hidden = np.asarray(decoder_hidden, dtype=np.float32)
    W_enc = np.asarray(W_enc, dtype=np.float32)
    W_dec = np.asarray(W_dec, dtype=np.float32)
    v = np.asarray(v, dtype=np.float32)

    nc = _get_nc()
    in_maps = _make_in_maps(encoder_out, decoder_hidden, W_enc, W_dec, v)
    res = run_bass_kernel_spmd(nc, in_maps, list(range(NCORES)))
    return _postprocess(res.results, mask)
